# revision 5
# baseline (speedup 1.0000x reference)
"""Trainium2 Bass kernel for nn_BlurF: depthwise 4x4 blur (upfirdn2d pad=(2,1)).

Fast path (blur kernel [1,3,3,1]x[1,3,3,1]/16, which is what setup_inputs
produces): hybrid split by output row to balance engines against the
~312 GB/s per-core DMA roofline (33.6 MB fp16 in+out => ~108 us floor).

  - Rows [0,192): PE banded-matmul path. Separable conv as two PE passes
    with the data stationary; band matrices hold integer taps [1,3,3,1]
    (input is host-prescaled by 1/16, exact in fp16). PSUM banks pack 2
    channels per bank (pass2 q1 packs 4 via tile_position col-tiling) so
    the PSUM->SBUF fp16 copies amortize per-op overhead; copies split
    between DVE and ACT.
  - Rows [192,256): elementwise path. Channels in partitions, both spatial
    dims free: [1,3,3,1] = [1,1]*[1,1]*[1,1], so 3 vertical + 3 horizontal
    shifted tensor_adds on DVE (fp16 2x mode).

General kernels fall back to the SVD banded path (previous baseline).
"""

import numpy as np
import concourse.bacc as bacc
import concourse.mybir as mybir
from concourse.tile import TileContext
from concourse.bass_utils import run_bass_kernel_spmd

N_CORES = 8
C, H, W = 128, 256, 256
PRECISION = "fp16io"  # fallback path precision
R_PE = 192            # rows on the PE path; rows [R_PE, 256) on the EW path

_BUILD_CACHE = {}


# ---------------------------------------------------------------- fast path
K1 = np.array([1.0, 3.0, 3.0, 1.0])


def _is_blur(kern):
    k2d = np.outer(K1, K1) / 16.0
    return kern.shape == (4, 4) and np.allclose(kern, k2d, rtol=0, atol=1e-6)


def _band(taps, n):
    """B[s, s'] = taps[a] where s' = s + a - 1, a in 0..3, clipped to [0,n)."""
    B = np.zeros((n, n), dtype=np.float64)
    for a in range(4):
        lo = max(0, 1 - a)
        hi = min(n, n + 1 - a)
        s = np.arange(lo, hi)
        B[s, s + a - 1] = taps[a]
    return B


def _fast_bands():
    """Pack band slices into one [128, 770] f16 tensor.

    cols [0:192)   bvt0 = B[y 0:128,   y' 0:192)
    cols [192:258) bvt1 = B[y 128:194, y' 126:192) on partitions 0:66
    cols [258:514) bht0 = B[x 0:128,   x' 0:256)
    cols [514:770) bht1 = B[x 128:256, x' 0:256)
    """
    B = _band(K1, 256)
    bd = np.zeros((128, 770), dtype=np.float32)
    bd[:, 0:192] = B[0:128, 0:192]
    bd[0:66, 192:258] = B[128:194, 126:192]
    bd[:, 258:514] = B[0:128, 0:256]
    bd[:, 514:770] = B[128:256, 0:256]
    return bd.astype(np.float16)


def _emit_fast(nc, tc, x, y, bd):
    f16 = mybir.dt.float16
    f32 = mybir.dt.float32
    G = 8          # channels per PE group
    NG = C // G
    with (
        tc.tile_pool(name="bands", bufs=1) as band_pool,
        tc.tile_pool(name="xin0", bufs=2) as xin0_pool,
        tc.tile_pool(name="xin1", bufs=2) as xin1_pool,
        tc.tile_pool(name="vts", bufs=4) as vt_pool,
        tc.tile_pool(name="yout0", bufs=2) as yout0_pool,
        tc.tile_pool(name="yout1", bufs=2) as yout1_pool,
        tc.tile_pool(name="p1", bufs=3, space="PSUM") as p1_pool,
        tc.tile_pool(name="p2a", bufs=2, space="PSUM") as p2a_pool,
        tc.tile_pool(name="p2b", bufs=2, space="PSUM") as p2b_pool,
        tc.tile_pool(name="ewin", bufs=2) as ewin_pool,
        tc.tile_pool(name="ewv", bufs=1) as ewv_pool,
        tc.tile_pool(name="ewh", bufs=1) as ewh_pool,
        tc.tile_pool(name="ewout", bufs=2) as ewout_pool,
    ):
        bds = band_pool.tile([128, 770], f16, tag="bd")
        nc.sync.dma_start(out=bds[:], in_=bd[:, :])
        bvt0 = bds[:, 0:192]
        bvt1 = bds[0:66, 192:258]
        bht = (bds[:, 258:514], bds[:, 514:770])

        def emit_ew_slab(s):
            o0 = R_PE + 32 * s
            tin = ewin_pool.tile([128, 35, 260], f16, tag="ewin")
            nc.gpsimd.memset(tin[:, :, 0:2], 0.0)
            nc.gpsimd.memset(tin[:, :, 258:260], 0.0)
            nrows = min(256, o0 + 33) - (o0 - 2)
            if nrows < 35:
                nc.gpsimd.memset(tin[:, nrows:35, :], 0.0)
            nc.sync.dma_start(out=tin[:, 0:nrows, 2:258],
                              in_=x[:, o0 - 2:o0 - 2 + nrows, :])
            v1 = ewv_pool.tile([128, 34, 260], f16, tag="v1")
            nc.vector.tensor_add(v1[:], tin[:, 0:34, :], tin[:, 1:35, :])
            v2 = ewv_pool.tile([128, 33, 260], f16, tag="v2")
            nc.vector.tensor_add(v2[:], v1[:, 0:33, :], v1[:, 1:34, :])
            v3 = ewv_pool.tile([128, 32, 260], f16, tag="v3")
            nc.vector.tensor_add(v3[:], v2[:, 0:32, :], v2[:, 1:33, :])
            h1 = ewh_pool.tile([128, 32, 258], f16, tag="h1")
            nc.vector.tensor_add(h1[:], v3[:, :, 0:258], v3[:, :, 1:259])
            h2 = ewh_pool.tile([128, 32, 257], f16, tag="h2")
            nc.vector.tensor_add(h2[:], h1[:, :, 0:257], h1[:, :, 1:258])
            oew = ewout_pool.tile([128, 32, 256], f16, tag="oew")
            nc.vector.tensor_add(oew[:], h2[:, :, 0:256], h2[:, :, 1:257])
            nc.gpsimd.dma_start(out=y[:, o0:o0 + 32, :], in_=oew[:])

        for g in range(NG):
            c0 = g * G
            xin0 = xin0_pool.tile([128, G, 256], f16, tag="xin0")
            xin1 = xin1_pool.tile([66, G, 256], f16, tag="xin1")
            for h in (0, 1):
                cs = c0 + h * (G // 2)
                nc.sync.dma_start(
                    out=xin0[:, h * (G // 2):(h + 1) * (G // 2), :],
                    in_=x[cs:cs + G // 2, 0:128, :].rearrange("c y x -> y c x"))
            nc.sync.dma_start(
                out=xin1[:],
                in_=x[c0:c0 + G, 128:194, :].rearrange("c y x -> y c x"))
            yout0 = yout0_pool.tile([128, G, 256], f16, tag="yout0")
            yout1 = yout1_pool.tile([128, G // 2, 256], f16, tag="yout1")
            p2q1 = [None]
            for p in range(G // 2):  # channel pairs
                j0 = 2 * p
                vts = []
                for m in (0, 1):
                    p1 = p1_pool.tile([128, 384], f32, tag="p1")
                    for cc in (0, 1):
                        j = j0 + cc
                        nc.tensor.matmul(
                            p1[:, cc * 192:(cc + 1) * 192],
                            xin0[:, j, m * 128:(m + 1) * 128],
                            bvt0, start=True, stop=False)
                        nc.tensor.matmul(
                            p1[:, cc * 192 + 126:cc * 192 + 192],
                            xin1[:, j, m * 128:(m + 1) * 128],
                            bvt1, start=False, stop=True)
                    v = vt_pool.tile([128, 384], f16, tag=f"vt{m}")
                    if m == 0:
                        nc.vector.tensor_copy(v[:], p1[:])
                    else:
                        nc.scalar.copy(v[:], p1[:])
                    vts.append(v)
                # pass2 q0: y' rows [0,128)
                p2 = p2a_pool.tile([128, 512], f32, tag="p2")
                for cc in (0, 1):
                    for m in (0, 1):
                        nc.tensor.matmul(
                            p2[:, cc * 256:(cc + 1) * 256],
                            vts[m][:, cc * 192:cc * 192 + 128],
                            bht[m], start=(m == 0), stop=(m == 1))
                nc.scalar.copy(yout0[:, j0:j0 + 2, :], p2[:])
                # pass2 q1: y' rows [128,192): pack 2 pairs via col-tiling
                po = 64 * (p % 2)
                if p % 2 == 0:
                    p2q1[0] = p2b_pool.tile([128, 512], f32, tag="p2q1", name="p2q1")
                q1 = p2q1[0]
                for cc in (0, 1):
                    for m in (0, 1):
                        nc.tensor.matmul(
                            q1[po:po + 64, cc * 256:(cc + 1) * 256],
                            vts[m][:, cc * 192 + 128:cc * 192 + 192],
                            bht[m], start=(m == 0), stop=(m == 1),
                            tile_position=(0, po))
                if p % 2 == 1:
                    b = p // 2
                    nc.vector.tensor_copy(
                        yout1[:, b * 2:(b + 1) * 2, :], q1[:])
            # out-DMAs for the group
            for h in (0, 1):
                cs = c0 + h * (G // 2)
                nc.gpsimd.dma_start(
                    out=y[cs:cs + G // 2, 0:128, :].rearrange("c y x -> y c x"),
                    in_=yout0[:, h * (G // 2):(h + 1) * (G // 2), :])
            # yout1 layout: partition<64: ch 4b+s (y' 128..191), >=64: ch 4b+2+s
            for b in range(G // 4):
                for s_ in (0, 1):
                    nc.gpsimd.dma_start(
                        out=y[c0 + 4 * b + s_, 128:192, :],
                        in_=yout1[0:64, b * 2 + s_, :])
                    nc.gpsimd.dma_start(
                        out=y[c0 + 4 * b + 2 + s_, 128:192, :],
                        in_=yout1[64:128, b * 2 + s_, :])
            if g == NG // 2 - 1:
                emit_ew_slab(0)
            if g == NG - 1:
                emit_ew_slab(1)


def _build_fast(reps=1, loop_reps=None):
    key = ("fast", reps, loop_reps)
    if key in _BUILD_CACHE:
        return _BUILD_CACHE[key]
    f16 = mybir.dt.float16
    nc = bacc.Bacc("TRN2", target_bir_lowering=False, debug=False)
    x = nc.dram_tensor("x", [C, H, W], f16, kind="ExternalInput").ap()
    bd = nc.dram_tensor("bd", [128, 770], f16, kind="ExternalInput").ap()
    y = nc.dram_tensor("y", [C, H, W], f16, kind="ExternalOutput").ap()
    with TileContext(nc) as tc:
        if loop_reps is not None:
            with tc.For_i(0, loop_reps, 1):
                _emit_fast(nc, tc, x, y, bd)
        else:
            for _ in range(reps):
                _emit_fast(nc, tc, x, y, bd)
    nc.compile()
    _BUILD_CACHE[key] = nc
    return nc


def _prep_fast(fmap):
    bd = _fast_bands()
    in_maps = []
    for i in range(N_CORES):
        x16 = (np.ascontiguousarray(fmap[i], dtype=np.float32) * (1.0 / 16.0)
               ).astype(np.float16)
        in_maps.append({"x": x16, "bd": bd})
    return in_maps


# ------------------------------------------------------- fallback (SVD) path
def _round_f32r(a):
    b = np.ascontiguousarray(a, dtype=np.float32).view(np.uint32)
    b = (b + np.uint32(0x800)) & np.uint32(0xFFFFF000)
    return b.view(np.float32)


def _factorize(kernel4x4):
    k = np.asarray(kernel4x4, dtype=np.float64)
    U, S, Vt = np.linalg.svd(k)
    comps = []
    for r in range(4):
        if S[r] > 1e-9 * max(S[0], 1e-30):
            comps.append((U[:, r] * np.sqrt(S[r]), Vt[r, :] * np.sqrt(S[r])))
    return comps


DEFAULT_CFG = dict(
    G=16, out_engine="scalar", dma_split=2,
    xin_bufs=2, vt_bufs=3, yout_bufs=2, p1_bufs=3, p2_bufs=3,
)


def _emit(nc, tc, x, y, bvt, bht, rank, precision, cfg=None):
    cfg = {**DEFAULT_CFG, **(cfg or {})}
    Gc = cfg["G"]
    f32 = mybir.dt.float32
    f32r = mybir.dt.float32r
    mmdt = {"fp32": f32, "fp16": mybir.dt.float16,
            "fp16io": mybir.dt.float16}.get(precision, f32r)
    ydt = mybir.dt.float16 if precision == "fp16io" else f32
    split = precision == "fp32r_split"
    parts = (0, 1) if split else (0,)
    NG = C // Gc
    out_dma = nc.scalar if cfg["out_engine"] == "scalar" else nc.sync
    with (
        tc.tile_pool(name="xin", bufs=cfg["xin_bufs"]) as xin_pool,
        tc.tile_pool(name="vt", bufs=cfg["vt_bufs"]) as vt_pool,
        tc.tile_pool(name="yout", bufs=cfg["yout_bufs"]) as yout_pool,
        tc.tile_pool(name="p1", bufs=cfg["p1_bufs"], space="PSUM") as p1_pool,
        tc.tile_pool(name="p2", bufs=cfg["p2_bufs"], space="PSUM") as p2_pool,
    ):
        pending = [None]

        def emit_pass2(p):
            vts, youts, j, g = p
            ops = [(r, m, s) for r in range(rank) for m in (0, 1) for s in parts]
            for q in (0, 1):
                p2 = p2_pool.tile([128, 256], f32, tag="p2")
                for i, (r, m, s) in enumerate(ops):
                    nc.tensor.matmul(
                        p2[:],
                        vts[(r, m, s)][:, q * 128:(q + 1) * 128],
                        bht[r][m][:],
                        start=(i == 0),
                        stop=(i == len(ops) - 1),
                    )
                if q == 0:
                    nc.vector.tensor_copy(youts[q][:, j, :], p2[:])
                else:
                    nc.scalar.copy(youts[q][:, j, :], p2[:])
            ds = cfg["dma_split"]
            gsz = Gc // ds
            if (j + 1) % gsz == 0:
                h = (j + 1) // gsz - 1
                c0 = g * Gc + h * gsz
                for q in (0, 1):
                    out_dma.dma_start(
                        out=y[c0:c0 + gsz, q * 128:(q + 1) * 128, :]
                        .rearrange("c y x -> y c x"),
                        in_=youts[q][:, h * gsz:(h + 1) * gsz, :],
                    )

        for g in range(NG):
            xraw = []
            ds = cfg["dma_split"]
            gsz = Gc // ds
            for t in (0, 1):
                xt = xin_pool.tile([128, Gc, 256], f32 if split else mmdt,
                                   tag=f"xin{t}", name=f"xin{t}")
                for h in range(ds):
                    c0 = g * Gc + h * gsz
                    nc.sync.dma_start(
                        out=xt[:, h * gsz:(h + 1) * gsz, :],
                        in_=x[c0:c0 + gsz, t * 128:(t + 1) * 128, :]
                        .rearrange("c y x -> y c x"),
                    )
                xraw.append(xt)
            if split:
                xins = {}
                for t in (0, 1):
                    hi = xin_pool.tile([128, Gc, 256], f32r, tag=f"xhi{t}", name=f"xhi{t}")
                    nc.scalar.copy(hi[:], xraw[t][:])
                    lo = xin_pool.tile([128, Gc, 256], f32r, tag=f"xlo{t}", name=f"xlo{t}")
                    nc.vector.tensor_sub(lo[:], xraw[t][:], hi[:])
                    xins[(t, 0)] = hi
                    xins[(t, 1)] = lo
            else:
                xins = {(t, 0): xraw[t] for t in (0, 1)}
            youts = {
                q: yout_pool.tile([128, Gc, 256], ydt, tag=f"yout{q}", name=f"yout{q}")
                for q in (0, 1)
            }
            for j in range(Gc):
                vts = {}
                for m in (0, 1):
                    for r in range(rank):
                        p1 = p1_pool.tile([128, 256], f32, tag="p1")
                        mmops = [(t, s) for t in (0, 1) for s in parts]
                        for i, (t, s) in enumerate(mmops):
                            nc.tensor.matmul(
                                p1[:],
                                xins[(t, s)][:, j, m * 128:(m + 1) * 128],
                                bvt[r][t][:],
                                start=(i == 0),
                                stop=(i == len(mmops) - 1),
                            )
                        if split:
                            vhi = vt_pool.tile([128, 256], f32r,
                                               tag=f"vth{m}_{r}", name=f"vth{m}_{r}")
                            nc.scalar.copy(vhi[:], p1[:])
                            vlo = vt_pool.tile([128, 256], f32r,
                                               tag=f"vtl{m}_{r}", name=f"vtl{m}_{r}")
                            nc.vector.tensor_sub(vlo[:], p1[:], vhi[:])
                            vts[(r, m, 0)] = vhi
                            vts[(r, m, 1)] = vlo
                        else:
                            v = vt_pool.tile([128, 256], mmdt,
                                             tag=f"vt{m}_{r}", name=f"vt{m}_{r}")
                            if m == 0:
                                nc.vector.tensor_copy(v[:], p1[:])
                            else:
                                nc.scalar.copy(v[:], p1[:])
                            vts[(r, m, 0)] = v
                if pending[0] is not None:
                    emit_pass2(pending[0])
                pending[0] = (vts, youts, j, g)
        emit_pass2(pending[0])


def _build(rank, precision, reps=1, loop_reps=None, cfg=None):
    key = (rank, precision, reps, loop_reps,
           tuple(sorted((cfg or {}).items())))
    if key in _BUILD_CACHE:
        return _BUILD_CACHE[key]
    f32 = mybir.dt.float32
    mmdt = {"fp32": f32, "fp16": mybir.dt.float16,
            "fp16io": mybir.dt.float16}.get(precision, mybir.dt.float32r)
    xdt = f32 if precision in ("fp32", "fp32r_split") else mmdt
    ydt = mybir.dt.float16 if precision == "fp16io" else f32
    nc = bacc.Bacc("TRN2", target_bir_lowering=False, debug=False)
    x = nc.dram_tensor("x", [C, H, W], xdt, kind="ExternalInput").ap()
    bv = nc.dram_tensor("bv", [rank, 2, 128, 256], mmdt, kind="ExternalInput").ap()
    bh = nc.dram_tensor("bh", [rank, 2, 128, 256], mmdt, kind="ExternalInput").ap()
    y = nc.dram_tensor("y", [C, H, W], ydt, kind="ExternalOutput").ap()
    with TileContext(nc) as tc:
        with tc.tile_pool(name="bands", bufs=1) as band_pool:
            bvt = [[None, None] for _ in range(rank)]
            bht = [[None, None] for _ in range(rank)]
            for r in range(rank):
                for t in (0, 1):
                    bvt[r][t] = band_pool.tile([128, 256], mmdt, tag=f"bv{r}{t}", name=f"bv{r}{t}")
                    nc.sync.dma_start(out=bvt[r][t][:], in_=bv[r, t])
                    bht[r][t] = band_pool.tile([128, 256], mmdt, tag=f"bh{r}{t}", name=f"bh{r}{t}")
                    nc.sync.dma_start(out=bht[r][t][:], in_=bh[r, t])
            if loop_reps is not None:
                with tc.For_i(0, loop_reps, 1):
                    _emit(nc, tc, x, y, bvt, bht, rank, precision, cfg)
            else:
                for _ in range(reps):
                    _emit(nc, tc, x, y, bvt, bht, rank, precision, cfg)
    nc.compile()
    _BUILD_CACHE[key] = nc
    return nc


def _band_f(taps, n):
    return _band(taps, n)


def _prep_inputs(fmap, kernel4x4, precision):
    comps = _factorize(kernel4x4)
    rank = max(1, len(comps))
    while len(comps) < rank:
        comps.append((np.zeros(4), np.zeros(4)))
    bv = np.zeros((rank, 2, 128, 256), dtype=np.float32)
    bh = np.zeros((rank, 2, 128, 256), dtype=np.float32)
    for r, (u, v) in enumerate(comps):
        Bv = _band(u, H).astype(np.float32)
        Bh = _band(v, W).astype(np.float32)
        bv[r] = Bv.reshape(2, 128, 256)
        bh[r] = Bh.reshape(2, 128, 256)
    if precision in ("fp32r", "fp32r_split"):
        bv, bh = _round_f32r(bv), _round_f32r(bh)
    elif precision in ("fp16", "fp16io"):
        bv, bh = bv.astype(np.float16), bh.astype(np.float16)
    in_maps = []
    for i in range(N_CORES):
        shard = np.ascontiguousarray(fmap[i], dtype=np.float32)
        if precision == "fp32r":
            shard = _round_f32r(shard)
        elif precision in ("fp16", "fp16io"):
            shard = shard.astype(np.float16)
        in_maps.append({"x": shard, "bv": bv, "bh": bh})
    return rank, in_maps


def _run(nc, in_maps):
    last_err = None
    for _attempt in range(3):
        try:
            return run_bass_kernel_spmd(nc, in_maps, list(range(N_CORES)),
                                        trace=False)
        except Exception as e:
            last_err = e
            import time
            time.sleep(2.0)
    raise last_err


def kernel(fmap, kernel):
    fmap = np.asarray(fmap)
    kern = np.asarray(kernel)
    assert fmap.shape == (N_CORES, C, H, W), fmap.shape
    if _is_blur(kern):
        in_maps = _prep_fast(fmap)
        nc = _build_fast()
        res = _run(nc, in_maps)
    else:
        rank, in_maps = _prep_inputs(fmap, kern, PRECISION)
        nc = _build(rank, PRECISION)
        res = _run(nc, in_maps)
    out = np.stack([res.results[i]["y"] for i in range(N_CORES)], axis=0)
    return np.ascontiguousarray(out.astype(np.float32))


# revision 6
# speedup vs baseline: 1.5570x; 1.5570x over previous
"""Trainium2 Bass kernel for nn_BlurF: depthwise 4x4 blur (upfirdn2d pad=(2,1)).

Fast path (blur kernel [1,3,3,1]x[1,3,3,1]/16, which is what setup_inputs
produces): hybrid split by output row to balance engines against the
~312 GB/s per-core DMA roofline (33.6 MB fp16 in+out => ~108 us floor).

  - Rows [0,192): PE banded-matmul path. Separable conv as two PE passes
    with the data stationary; band matrices hold integer taps [1,3,3,1]
    (input is host-prescaled by 1/16, exact in fp16). PSUM banks pack 2
    channels per bank (pass2 q1 packs 4 via tile_position col-tiling) so
    the PSUM->SBUF fp16 copies amortize per-op overhead; copies split
    between DVE and ACT.
  - Rows [192,256): elementwise path. Channels in partitions, both spatial
    dims free: [1,3,3,1] = [1,1]*[1,1]*[1,1], so 3 vertical + 3 horizontal
    shifted tensor_adds on DVE (fp16 2x mode).

General kernels fall back to the SVD banded path (previous baseline).
"""

import numpy as np
import concourse.bacc as bacc
import concourse.mybir as mybir
from concourse.tile import TileContext
from concourse.bass_utils import run_bass_kernel_spmd

N_CORES = 8
C, H, W = 128, 256, 256
PRECISION = "fp16io"  # fallback path precision
R_PE = 192            # rows on the PE path; rows [R_PE, 256) on the EW path

_BUILD_CACHE = {}


# ---------------------------------------------------------------- fast path
K1 = np.array([1.0, 3.0, 3.0, 1.0])


def _is_blur(kern):
    k2d = np.outer(K1, K1) / 16.0
    return kern.shape == (4, 4) and np.allclose(kern, k2d, rtol=0, atol=1e-6)


def _band(taps, n):
    """B[s, s'] = taps[a] where s' = s + a - 1, a in 0..3, clipped to [0,n)."""
    B = np.zeros((n, n), dtype=np.float64)
    for a in range(4):
        lo = max(0, 1 - a)
        hi = min(n, n + 1 - a)
        s = np.arange(lo, hi)
        B[s, s + a - 1] = taps[a]
    return B


def _fast_bands():
    """Two band tiles in one [128, 512] f16 tensor.

    cols [0:256)   BD0 = B[0:128, 0:256]    (pass1 t=0 rhs; pass2 m=0 rhs)
    cols [256:512) BD1 = B[128:256, 0:256]  (pass2 m=1 rhs; [:,126:256] = pass1 t=1 rhs)
    """
    B = _band(K1, 256)
    bd = np.zeros((128, 512), dtype=np.float32)
    bd[:, 0:256] = B[0:128, 0:256]
    bd[:, 256:512] = B[128:256, 0:256]
    return bd.astype(np.float16)


FAST_CFG = dict(G=8, xin_bufs=2, vt_bufs=4, yout_bufs=2, p1_bufs=3,
                p2_bufs=4, out_engine="gpsimd", dma_split=2)


def _emit_fast(nc, tc, x, y, bd, cfg=None):
    cfg = {**FAST_CFG, **(cfg or {})}
    f16 = mybir.dt.float16
    f32 = mybir.dt.float32
    G = cfg["G"]
    NG = C // G
    out_dma = {"gpsimd": nc.gpsimd, "scalar": nc.scalar,
               "sync": nc.sync}[cfg["out_engine"]]
    with (
        tc.tile_pool(name="bands", bufs=1) as band_pool,
        tc.tile_pool(name="xin0", bufs=cfg["xin_bufs"]) as xin0_pool,
        tc.tile_pool(name="xin1", bufs=cfg["xin_bufs"]) as xin1_pool,
        tc.tile_pool(name="vts", bufs=cfg["vt_bufs"]) as vt_pool,
        tc.tile_pool(name="yout", bufs=cfg["yout_bufs"]) as yout_pool,
        tc.tile_pool(name="p1", bufs=cfg["p1_bufs"], space="PSUM") as p1_pool,
        tc.tile_pool(name="p2", bufs=cfg["p2_bufs"], space="PSUM") as p2_pool,
    ):
        bds = band_pool.tile([128, 512], f16, tag="bd")
        nc.sync.dma_start(out=bds[:], in_=bd[:, :])
        BD0 = bds[:, 0:256]
        BD1 = bds[:, 256:512]
        BD1c = bds[:, 256 + 126:512]

        for g in range(NG):
            c0 = g * G
            xins = []
            for t in (0, 1):
                xt = (xin0_pool if t == 0 else xin1_pool).tile(
                    [128, G, 256], f16, tag=f"xin{t}", name=f"xin{t}")
                for h in range(cfg["dma_split"]):
                    gsz = G // cfg["dma_split"]
                    cs = c0 + h * gsz
                    nc.sync.dma_start(
                        out=xt[:, h * gsz:(h + 1) * gsz, :],
                        in_=x[cs:cs + gsz, t * 128:(t + 1) * 128, :]
                        .rearrange("c y x -> y c x"))
                xins.append(xt)
            youts = [yout_pool.tile([128, G, 256], f16, tag=f"yout{q}",
                                    name=f"yout{q}") for q in (0, 1)]
            for p in range(G // 2):  # channel pairs
                j0 = 2 * p
                vts = []
                for m in (0, 1):
                    p1 = p1_pool.tile([128, 512], f32, tag="p1")
                    for cc in (0, 1):
                        j = j0 + cc
                        nc.tensor.matmul(
                            p1[:, cc * 256:(cc + 1) * 256],
                            xins[0][:, j, m * 128:(m + 1) * 128],
                            BD0, start=True, stop=False)
                        nc.tensor.matmul(
                            p1[:, cc * 256 + 126:(cc + 1) * 256],
                            xins[1][:, j, m * 128:(m + 1) * 128],
                            BD1c, start=False, stop=True)
                    v = vt_pool.tile([128, 512], f16, tag=f"vt{m}",
                                     name=f"vt{m}")
                    if m == 0:
                        nc.vector.tensor_copy(v[:], p1[:])
                    else:
                        nc.scalar.copy(v[:], p1[:])
                    vts.append(v)
                for q in (0, 1):
                    p2 = p2_pool.tile([128, 512], f32, tag="p2")
                    for cc in (0, 1):
                        for m in (0, 1):
                            nc.tensor.matmul(
                                p2[:, cc * 256:(cc + 1) * 256],
                                vts[m][:, cc * 256 + q * 128:
                                       cc * 256 + q * 128 + 128],
                                BD0 if m == 0 else BD1,
                                start=(m == 0), stop=(m == 1))
                    if q == 0:
                        nc.scalar.copy(youts[q][:, j0:j0 + 2, :], p2[:])
                    else:
                        nc.vector.tensor_copy(youts[q][:, j0:j0 + 2, :], p2[:])
            for q in (0, 1):
                for h in range(cfg["dma_split"]):
                    gsz = G // cfg["dma_split"]
                    cs = c0 + h * gsz
                    out_dma.dma_start(
                        out=y[cs:cs + gsz, q * 128:(q + 1) * 128, :]
                        .rearrange("c y x -> y c x"),
                        in_=youts[q][:, h * gsz:(h + 1) * gsz, :])


def _build_fast(reps=1, loop_reps=None, cfg=None):
    key = ("fast", reps, loop_reps, tuple(sorted((cfg or {}).items())))
    if key in _BUILD_CACHE:
        return _BUILD_CACHE[key]
    f16 = mybir.dt.float16
    nc = bacc.Bacc("TRN2", target_bir_lowering=False, debug=False)
    x = nc.dram_tensor("x", [C, H, W], f16, kind="ExternalInput").ap()
    bd = nc.dram_tensor("bd", [128, 512], f16, kind="ExternalInput").ap()
    y = nc.dram_tensor("y", [C, H, W], f16, kind="ExternalOutput").ap()
    with TileContext(nc) as tc:
        if loop_reps is not None:
            with tc.For_i(0, loop_reps, 1):
                _emit_fast(nc, tc, x, y, bd, cfg)
        else:
            for _ in range(reps):
                _emit_fast(nc, tc, x, y, bd, cfg)
    nc.compile()
    _BUILD_CACHE[key] = nc
    return nc


def _prep_fast(fmap):
    bd = _fast_bands()
    in_maps = []
    for i in range(N_CORES):
        x16 = (np.ascontiguousarray(fmap[i], dtype=np.float32) * (1.0 / 16.0)
               ).astype(np.float16)
        in_maps.append({"x": x16, "bd": bd})
    return in_maps


# ------------------------------------------------------- fallback (SVD) path
def _round_f32r(a):
    b = np.ascontiguousarray(a, dtype=np.float32).view(np.uint32)
    b = (b + np.uint32(0x800)) & np.uint32(0xFFFFF000)
    return b.view(np.float32)


def _factorize(kernel4x4):
    k = np.asarray(kernel4x4, dtype=np.float64)
    U, S, Vt = np.linalg.svd(k)
    comps = []
    for r in range(4):
        if S[r] > 1e-9 * max(S[0], 1e-30):
            comps.append((U[:, r] * np.sqrt(S[r]), Vt[r, :] * np.sqrt(S[r])))
    return comps


DEFAULT_CFG = dict(
    G=16, out_engine="scalar", dma_split=2,
    xin_bufs=2, vt_bufs=3, yout_bufs=2, p1_bufs=3, p2_bufs=3,
)


def _emit(nc, tc, x, y, bvt, bht, rank, precision, cfg=None):
    cfg = {**DEFAULT_CFG, **(cfg or {})}
    Gc = cfg["G"]
    f32 = mybir.dt.float32
    f32r = mybir.dt.float32r
    mmdt = {"fp32": f32, "fp16": mybir.dt.float16,
            "fp16io": mybir.dt.float16}.get(precision, f32r)
    ydt = mybir.dt.float16 if precision == "fp16io" else f32
    split = precision == "fp32r_split"
    parts = (0, 1) if split else (0,)
    NG = C // Gc
    out_dma = nc.scalar if cfg["out_engine"] == "scalar" else nc.sync
    with (
        tc.tile_pool(name="xin", bufs=cfg["xin_bufs"]) as xin_pool,
        tc.tile_pool(name="vt", bufs=cfg["vt_bufs"]) as vt_pool,
        tc.tile_pool(name="yout", bufs=cfg["yout_bufs"]) as yout_pool,
        tc.tile_pool(name="p1", bufs=cfg["p1_bufs"], space="PSUM") as p1_pool,
        tc.tile_pool(name="p2", bufs=cfg["p2_bufs"], space="PSUM") as p2_pool,
    ):
        pending = [None]

        def emit_pass2(p):
            vts, youts, j, g = p
            ops = [(r, m, s) for r in range(rank) for m in (0, 1) for s in parts]
            for q in (0, 1):
                p2 = p2_pool.tile([128, 256], f32, tag="p2")
                for i, (r, m, s) in enumerate(ops):
                    nc.tensor.matmul(
                        p2[:],
                        vts[(r, m, s)][:, q * 128:(q + 1) * 128],
                        bht[r][m][:],
                        start=(i == 0),
                        stop=(i == len(ops) - 1),
                    )
                if q == 0:
                    nc.vector.tensor_copy(youts[q][:, j, :], p2[:])
                else:
                    nc.scalar.copy(youts[q][:, j, :], p2[:])
            ds = cfg["dma_split"]
            gsz = Gc // ds
            if (j + 1) % gsz == 0:
                h = (j + 1) // gsz - 1
                c0 = g * Gc + h * gsz
                for q in (0, 1):
                    out_dma.dma_start(
                        out=y[c0:c0 + gsz, q * 128:(q + 1) * 128, :]
                        .rearrange("c y x -> y c x"),
                        in_=youts[q][:, h * gsz:(h + 1) * gsz, :],
                    )

        for g in range(NG):
            xraw = []
            ds = cfg["dma_split"]
            gsz = Gc // ds
            for t in (0, 1):
                xt = xin_pool.tile([128, Gc, 256], f32 if split else mmdt,
                                   tag=f"xin{t}", name=f"xin{t}")
                for h in range(ds):
                    c0 = g * Gc + h * gsz
                    nc.sync.dma_start(
                        out=xt[:, h * gsz:(h + 1) * gsz, :],
                        in_=x[c0:c0 + gsz, t * 128:(t + 1) * 128, :]
                        .rearrange("c y x -> y c x"),
                    )
                xraw.append(xt)
            if split:
                xins = {}
                for t in (0, 1):
                    hi = xin_pool.tile([128, Gc, 256], f32r, tag=f"xhi{t}", name=f"xhi{t}")
                    nc.scalar.copy(hi[:], xraw[t][:])
                    lo = xin_pool.tile([128, Gc, 256], f32r, tag=f"xlo{t}", name=f"xlo{t}")
                    nc.vector.tensor_sub(lo[:], xraw[t][:], hi[:])
                    xins[(t, 0)] = hi
                    xins[(t, 1)] = lo
            else:
                xins = {(t, 0): xraw[t] for t in (0, 1)}
            youts = {
                q: yout_pool.tile([128, Gc, 256], ydt, tag=f"yout{q}", name=f"yout{q}")
                for q in (0, 1)
            }
            for j in range(Gc):
                vts = {}
                for m in (0, 1):
                    for r in range(rank):
                        p1 = p1_pool.tile([128, 256], f32, tag="p1")
                        mmops = [(t, s) for t in (0, 1) for s in parts]
                        for i, (t, s) in enumerate(mmops):
                            nc.tensor.matmul(
                                p1[:],
                                xins[(t, s)][:, j, m * 128:(m + 1) * 128],
                                bvt[r][t][:],
                                start=(i == 0),
                                stop=(i == len(mmops) - 1),
                            )
                        if split:
                            vhi = vt_pool.tile([128, 256], f32r,
                                               tag=f"vth{m}_{r}", name=f"vth{m}_{r}")
                            nc.scalar.copy(vhi[:], p1[:])
                            vlo = vt_pool.tile([128, 256], f32r,
                                               tag=f"vtl{m}_{r}", name=f"vtl{m}_{r}")
                            nc.vector.tensor_sub(vlo[:], p1[:], vhi[:])
                            vts[(r, m, 0)] = vhi
                            vts[(r, m, 1)] = vlo
                        else:
                            v = vt_pool.tile([128, 256], mmdt,
                                             tag=f"vt{m}_{r}", name=f"vt{m}_{r}")
                            if m == 0:
                                nc.vector.tensor_copy(v[:], p1[:])
                            else:
                                nc.scalar.copy(v[:], p1[:])
                            vts[(r, m, 0)] = v
                if pending[0] is not None:
                    emit_pass2(pending[0])
                pending[0] = (vts, youts, j, g)
        emit_pass2(pending[0])


def _build(rank, precision, reps=1, loop_reps=None, cfg=None):
    key = (rank, precision, reps, loop_reps,
           tuple(sorted((cfg or {}).items())))
    if key in _BUILD_CACHE:
        return _BUILD_CACHE[key]
    f32 = mybir.dt.float32
    mmdt = {"fp32": f32, "fp16": mybir.dt.float16,
            "fp16io": mybir.dt.float16}.get(precision, mybir.dt.float32r)
    xdt = f32 if precision in ("fp32", "fp32r_split") else mmdt
    ydt = mybir.dt.float16 if precision == "fp16io" else f32
    nc = bacc.Bacc("TRN2", target_bir_lowering=False, debug=False)
    x = nc.dram_tensor("x", [C, H, W], xdt, kind="ExternalInput").ap()
    bv = nc.dram_tensor("bv", [rank, 2, 128, 256], mmdt, kind="ExternalInput").ap()
    bh = nc.dram_tensor("bh", [rank, 2, 128, 256], mmdt, kind="ExternalInput").ap()
    y = nc.dram_tensor("y", [C, H, W], ydt, kind="ExternalOutput").ap()
    with TileContext(nc) as tc:
        with tc.tile_pool(name="bands", bufs=1) as band_pool:
            bvt = [[None, None] for _ in range(rank)]
            bht = [[None, None] for _ in range(rank)]
            for r in range(rank):
                for t in (0, 1):
                    bvt[r][t] = band_pool.tile([128, 256], mmdt, tag=f"bv{r}{t}", name=f"bv{r}{t}")
                    nc.sync.dma_start(out=bvt[r][t][:], in_=bv[r, t])
                    bht[r][t] = band_pool.tile([128, 256], mmdt, tag=f"bh{r}{t}", name=f"bh{r}{t}")
                    nc.sync.dma_start(out=bht[r][t][:], in_=bh[r, t])
            if loop_reps is not None:
                with tc.For_i(0, loop_reps, 1):
                    _emit(nc, tc, x, y, bvt, bht, rank, precision, cfg)
            else:
                for _ in range(reps):
                    _emit(nc, tc, x, y, bvt, bht, rank, precision, cfg)
    nc.compile()
    _BUILD_CACHE[key] = nc
    return nc


def _band_f(taps, n):
    return _band(taps, n)


def _prep_inputs(fmap, kernel4x4, precision):
    comps = _factorize(kernel4x4)
    rank = max(1, len(comps))
    while len(comps) < rank:
        comps.append((np.zeros(4), np.zeros(4)))
    bv = np.zeros((rank, 2, 128, 256), dtype=np.float32)
    bh = np.zeros((rank, 2, 128, 256), dtype=np.float32)
    for r, (u, v) in enumerate(comps):
        Bv = _band(u, H).astype(np.float32)
        Bh = _band(v, W).astype(np.float32)
        bv[r] = Bv.reshape(2, 128, 256)
        bh[r] = Bh.reshape(2, 128, 256)
    if precision in ("fp32r", "fp32r_split"):
        bv, bh = _round_f32r(bv), _round_f32r(bh)
    elif precision in ("fp16", "fp16io"):
        bv, bh = bv.astype(np.float16), bh.astype(np.float16)
    in_maps = []
    for i in range(N_CORES):
        shard = np.ascontiguousarray(fmap[i], dtype=np.float32)
        if precision == "fp32r":
            shard = _round_f32r(shard)
        elif precision in ("fp16", "fp16io"):
            shard = shard.astype(np.float16)
        in_maps.append({"x": shard, "bv": bv, "bh": bh})
    return rank, in_maps


def _run(nc, in_maps):
    last_err = None
    for _attempt in range(3):
        try:
            return run_bass_kernel_spmd(nc, in_maps, list(range(N_CORES)),
                                        trace=False)
        except Exception as e:
            last_err = e
            import time
            time.sleep(2.0)
    raise last_err


def kernel(fmap, kernel):
    fmap = np.asarray(fmap)
    kern = np.asarray(kernel)
    assert fmap.shape == (N_CORES, C, H, W), fmap.shape
    if _is_blur(kern):
        in_maps = _prep_fast(fmap)
        nc = _build_fast()
        res = _run(nc, in_maps)
    else:
        rank, in_maps = _prep_inputs(fmap, kern, PRECISION)
        nc = _build(rank, PRECISION)
        res = _run(nc, in_maps)
    out = np.stack([res.results[i]["y"] for i in range(N_CORES)], axis=0)
    return np.ascontiguousarray(out.astype(np.float32))
